# revision 14
# baseline (speedup 1.0000x reference)
"""Patch-orthogonal-mix (unfold -> [L,D]@[D,D]^T -> fold) on 8 Trainium2 NeuronCores.

Strategy: pure data parallel over batch (2 images per core), weights replicated.
Per core, each image is processed in horizontal strips (16 pixel rows for the
first/last two, 32 for the rest; a small first strip shortens pipeline fill and
a small last strip shortens the drain tail).

The unfold is realized by the input DMA layout: SBUF x-tiles hold partitions
p = ph_off*64 + c (ph_off = patch-row offset within a row-pair, c = channel),
so the patch-vector contraction dim d = (c, ph, pw) maps onto matmul
K-partitions, with full-resolution rows loaded contiguously (1KB runs, no
data duplication) and cast f32->f16 inside the SWDGE DMA. One DMA per x-tile
(the partition axis composes (ph_off, c) from two DRAM strides), halving the
serial Q7 descriptor-emission cost per strip.

Mixed-precision contraction: of the 8 K-chunks (a = row-pair, pw = in-patch
column), the two pw==0 chunks are computed in fp8-e4m3 through a single
DoubleRow matmul (the PE contracts both chunks in one pass at 2x MAC rate,
both operands e4m3), and the remaining 6 chunks in fp16: 6 fp16 + 1 DoubleRow
= measured ~1526ns per 7-matmul output group vs 1728ns for 8 fp16 passes
(11.7% less PE time) at a measured 1.70e-2 relative error. All weights are
host-packed at 32x scale so the e4m3 copy of W stays in its normal range; the
PSUM->SBUF copies then apply the exact 1/32 scale for free.

Schedule (from trace analysis of the previous version, which lost ~50us to a
startup convoy):
  * Weights are packed m-major (output-tile-major) and loaded as 4 big HWDGE
    DMAs on the sync ring + 1 on the scalar ring, instead of 28 64KB DMAs
    that serialized ~25us on one FIFO and stalled LDWEIGHTS until ts~54us.
  * ~20 warmup matmuls on a zeroed tile run while the first data loads, so
    the PE's HAM clock-gate reaches K=8/8 (2.4 GHz) before the first real
    matmul and the fill phase doesn't pay the 1.2 GHz cold clock.
  * Output DMAs alternate between the sync and scalar HWDGE rings (one DMA
    per 64-row-pair, composite partition axis) so outputs never queue behind
    weights and drain in parallel at the tail.

A DVE copy gathers the stride-4 pw columns of the fp16 x-tiles into
contiguous blocks (the PE streams contiguous operands at 1 col/cycle but
pays ~2x for strided), and the same gather with an e4m3 output dtype
produces the DoubleRow moving operand. fp32 PSUM accumulation; the fold is
realized by stride-4 interleaving scaled PSUM->SBUF copies (alternating
scalar/vector engines) plus a mirrored output DMA pattern.
"""
import numpy as np
import ml_dtypes

import concourse.bass as bass
import concourse.bacc as bacc
import concourse.mybir as mybir
from concourse.tile import TileContext
from concourse.bass_utils import run_bass_kernel_spmd

P = 4
C = 64
H = W = 256
B = 16
N_CORES = 8
B_LOC = B // N_CORES          # batches per core
WP = W // P                   # patch-cols (64)
F32 = mybir.dt.float32
F16 = mybir.dt.float16
F8 = mybir.dt.float8e4
DR = mybir.MatmulPerfMode.DoubleRow
OSCALE = 1.0 / 32.0
N_WARM = 10


def _build():
    nc = bacc.Bacc()
    x = nc.declare_dram_parameter("x", [B_LOC, C, H, W], F32, isOutput=False)
    w16 = nc.declare_dram_parameter("w16", [128, 6144], F16, isOutput=False)
    w8 = nc.declare_dram_parameter("w8", [128, 2048], F8, isOutput=False)
    y = nc.declare_dram_parameter("y", [B_LOC, C, H, W], F32, isOutput=True)

    with TileContext(nc) as tc:
        with (
            tc.tile_pool(name="wpool", bufs=1) as wpool,
            tc.tile_pool(name="wupool", bufs=1) as wupool,
            tc.tile_pool(name="xpool", bufs=10) as xpool,
            tc.tile_pool(name="gpool", bufs=10) as gpool,
            tc.tile_pool(name="g8pool", bufs=5) as g8pool,
            tc.tile_pool(name="spool", bufs=8) as spool,
            tc.tile_pool(name="psum", bufs=8, space="PSUM") as ppool,
        ):
            # Weights m-major: w16 column j = ((m*6 + a*3 + pwi)*128 + (php,c')
            # so each output group's 6 fp16 chunks are contiguous.  Four big
            # DMAs on the sync HWDGE ring (m pairs), w8 on the scalar ring.
            wt = wpool.tile([128, 6144], F16, tag="w")
            w8t = wpool.tile([128, 2048], F8, tag="w8")
            # Weight DMAs sliced to 512-col chunks (1KB descriptors, same size
            # as the x-input descriptors) and split across both HWDGE rings:
            # the SDMA engines round-robin rings at packet granularity, so
            # equal descriptor sizes keep the early bandwidth split fair
            # between weights and the x input stream (big-descriptor weight
            # DMAs measurably starve the input 5:1 during pipeline fill).
            # Emission order = first-use order (weights are packed m-major).
            nc.scalar.dma_start(out=w8t[:, 0:1024], in_=w8[:, 0:1024])
            for j in range(12):
                eng = nc.sync if j % 2 == 0 else nc.scalar
                eng.dma_start(out=wt[:, j * 512:(j + 1) * 512],
                              in_=w16[:, j * 512:(j + 1) * 512])
                if j == 3:
                    # second half of w8 (DR chunks for m4-m7) mid-stream
                    nc.scalar.dma_start(out=w8t[:, 1024:2048],
                                        in_=w8[:, 1024:2048])
            w8v = w8t[:].rearrange("p (m a f) -> p m a f", m=8, a=2)

            # Warmup: dummy matmuls on a zeroed tile keep the PE busy while
            # the first weights/x land, so HAM un-throttles to 2.4 GHz before
            # real work starts.  The scratch PSUM tile is never read.
            wu = wupool.tile([128, 512], F16, tag="wu")
            nc.vector.memset(wu[:], 0.0)
            wps = ppool.tile([128, 512], F32, tag="ps", name="warm_ps")
            for k in range(N_WARM):
                nc.tensor.matmul(wps[:], lhsT=wu[:, :128], rhs=wu[:],
                                 start=(k == 0), stop=(k == N_WARM - 1))

            strips = ([(0, 0, 16), (0, 16, 16)]
                      + [(0, r, 32) for r in range(32, 256, 32)]
                      + [(1, r, 32) for r in range(0, 224, 32)]
                      + [(1, 224, 16), (1, 240, 8), (1, 248, 8)])

            for si, (b, r0, rows) in enumerate(strips):
                hp_s = rows // P
                n_l = hp_s * WP
                # rows of the strip grouped by h%4: [ph, c, hp, w]
                src4 = x[b, :, r0:r0 + rows, :].rearrange(
                    "c (hp ph) w -> ph c hp w", ph=P)
                xg = []
                ts = []
                for a in range(2):
                    t = xpool.tile([128, hp_s * 256], F16, tag="x")
                    for ph_off in range(2):
                        dst = t[ph_off * 64:(ph_off + 1) * 64, :].rearrange(
                            "p (hp w) -> p hp w", w=256)
                        # f32 -> f16 cast happens in the DMA (SWDGE only)
                        nc.gpsimd.dma_start(out=dst, in_=src4[2 * a + ph_off])
                    ts.append(t)
                    # gather pw-strided columns (pw in {1,2,3}) into contiguous
                    # fp16 blocks so the matmul rhs streams at 1 col/cycle
                    g = gpool.tile([128, hp_s * 192], F16, tag="xg")
                    nc.vector.tensor_copy(
                        out=g[:].rearrange("p (pw hp wp) -> p pw hp wp",
                                           hp=hp_s, wp=WP),
                        in_=t[:].rearrange("p (hp wp pw) -> p pw hp wp",
                                           wp=WP, pw=P)[:, 1:4],
                    )
                    xg.append(g)
                # pw==0 columns of both row-pairs, cast f16 -> e4m3: the
                # DoubleRow moving operand [128, 2, n_l]
                g8 = g8pool.tile([128, 2 * n_l], F8, tag="x8")
                for a in range(2):
                    nc.vector.tensor_copy(
                        out=g8[:, a * n_l:(a + 1) * n_l].rearrange(
                            "p (hp wp) -> p hp wp", wp=WP),
                        in_=ts[a][:].rearrange("p (hp wp pw) -> p hp wp pw",
                                               wp=WP, pw=P)[:, :, :, 0],
                    )
                g8r = g8[:].rearrange("p (a n) -> p a n", a=2)
                xr = [[g[:, pwi * n_l:(pwi + 1) * n_l] for pwi in range(3)]
                      for g in xg]

                dsty4 = y[b, :, r0:r0 + rows, :].rearrange(
                    "c (hp ph) w -> ph c hp w", ph=P)
                for b2 in range(2):
                    st = spool.tile([128, hp_s * 256], F32, tag="st")
                    st_r = st[:].rearrange("p (hp wp pw) -> pw p (hp wp)",
                                           wp=WP, pw=P)
                    for pwp in range(P):
                        m_idx = b2 * P + pwp
                        ps = ppool.tile([128, n_l], F32)
                        # DoubleRow mid-group: both group boundaries stay
                        # fp16<->fp16 (cheap), and start/stop stay on fp16
                        # matmuls (start=True on a DoubleRow measurably
                        # degrades accuracy on hardware)
                        step = 0
                        for a in range(2):
                            for pwi in range(3):
                                f0 = (m_idx * 6 + a * 3 + pwi) * 128
                                nc.tensor.matmul(
                                    ps[:],
                                    lhsT=wt[:, f0:f0 + 128],
                                    rhs=xr[a][pwi],
                                    start=(step == 0),
                                    stop=(step == 5),
                                )
                                step += 1
                            if a == 0:
                                nc.tensor.matmul(
                                    ps[:],
                                    lhsT=w8v[:, m_idx],
                                    rhs=g8r,
                                    start=False,
                                    stop=False,
                                    perf_mode=DR,
                                )
                        if pwp % 2 == 0:
                            nc.scalar.mul(out=st_r[pwp], in_=ps[:], mul=OSCALE)
                        else:
                            nc.vector.tensor_scalar_mul(out=st_r[pwp],
                                                        in0=ps[:],
                                                        scalar1=OSCALE)
                    # output DMAs alternate between the two HWDGE rings
                    for php_off in range(2):
                        srcs = st[php_off * 64:(php_off + 1) * 64, :].rearrange(
                            "p (hp w) -> p hp w", w=256)
                        eng = nc.sync if b2 == 0 else nc.scalar
                        eng.dma_start(out=dsty4[2 * b2 + php_off], in_=srcs)
    nc.compile()
    return nc


def _pack_w(W_mat):
    # All weights packed at 32x so the e4m3 copy sits in its normal range;
    # the PSUM->SBUF copies divide by 32 (exact).
    # lhsT partitions p = ph_off*64 + c over the d-chunk
    # d = c*16 + (2a+ph_off)*4 + pw; e = c'*16 + (2*b2+php_off)*4 + pwp.
    W32 = np.asarray(W_mat, dtype=np.float32) * np.float32(32.0)
    Wr = W32.reshape(64, 2, 2, 4, 64, 2, 2, 4)
    # axes in: (c', b2, php_off, pwp, c, a, ph_off, pw)
    Wp = Wr.transpose(6, 4, 1, 3, 5, 7, 2, 0)
    # -> (ph_off, c, b2, pwp, a, pw, php_off, c')   [m-major columns]
    w16 = np.ascontiguousarray(
        Wp[:, :, :, :, :, 1:4].reshape(128, 6144).astype(np.float16))
    w8 = np.ascontiguousarray(
        Wp[:, :, :, :, :, 0].reshape(128, 2048).astype(ml_dtypes.float8_e4m3fn))
    return w16, w8


_nc_cache = None


def _get_nc():
    global _nc_cache
    if _nc_cache is None:
        _nc_cache = _build()
    return _nc_cache


def _run(x, W_mat, trace=False, **kwargs):
    x = np.ascontiguousarray(np.asarray(x, dtype=np.float32))
    w16, w8 = _pack_w(W_mat)
    nc = _get_nc()
    in_maps = [
        {"x": np.ascontiguousarray(x[i * B_LOC:(i + 1) * B_LOC]),
         "w16": w16, "w8": w8}
        for i in range(N_CORES)
    ]
    res = run_bass_kernel_spmd(nc, in_maps, list(range(N_CORES)), trace=trace,
                               **kwargs)
    y = np.concatenate([np.asarray(res.results[i]["y"]) for i in range(N_CORES)],
                       axis=0)
    return y, res


def kernel(**inputs):
    y, _ = _run(inputs["x"], inputs["W_mat"])
    return y


# revision 15
# speedup vs baseline: 1.0510x; 1.0510x over previous
"""Patch-orthogonal-mix (unfold -> [L,D]@[D,D]^T -> fold) on 8 Trainium2 NeuronCores.

Strategy: pure data parallel over batch (2 images per core), weights replicated.
Per core, each image is processed in horizontal strips (16 pixel rows for the
first two / last few, 32 for the rest; small edge strips shorten pipeline fill
and drain).

The unfold is done ON THE HOST during input packing: x is cast f32->f16 (the
same RNE rounding the previous in-DMA cast applied) and laid out per-strip in
exactly the SBUF tile format the matmuls consume:
  * xg  [128, .] f16 : partitions p = ph_off*64 + c, free dim (a, pw-1, hp, wp)
        for the six pw!=0 K-chunks (a = row-pair, pw = in-patch column),
  * x8  [128, .] e4m3: the two pw==0 chunks, the DoubleRow moving operand.
This more than halves input HBM traffic (14.7MB vs 33.5MB f32 per core) and
removes the on-device DVE gather stage entirely.  The output is written f16
(16.8MB vs 33.5MB) and upcast to f32 on the host -- the f16 rounding adds
~2.9e-4 relative error in quadrature, invisible next to the 1.70e-2 fp8 term.
With both changes the kernel runs far below the ~358 GB/s HBM-per-core limit
(measured pegged at ~350 with f32 I/O, which stalled the PE mid-run) and is
purely PE-bound.

Mixed-precision contraction: of the 8 K-chunks, the two pw==0 chunks are
computed in fp8-e4m3 through a single DoubleRow matmul (2x MAC rate), the
remaining 6 in fp16: measured ~1526ns per 7-matmul output group vs 1728ns for
8 fp16 passes at a 1.70e-2 relative error.  All weights are host-packed at
32x scale so the e4m3 copy of W stays in its normal range; the PSUM->SBUF
copies then apply the exact 1/32 scale for free.

Schedule: weights are packed m-major (output-tile-major) and sliced into
512-col DMAs (1KB descriptors, same as the input stream's) alternating
between the sync and scalar HWDGE rings -- equal descriptor sizes keep the
SDMA packet round-robin fair so the early weight burst cannot starve the
input stream.  ~10 warmup matmuls on a zeroed tile run while the first data
loads so the PE's HAM clock-gate reaches K=8/8 (2.4 GHz) before real work.
Output DMAs alternate rings; fp32 PSUM accumulation; the fold is realized by
stride-4 interleaving scaled PSUM->SBUF copies (alternating scalar/vector
engines) plus the output DMA pattern.
"""
import numpy as np
import ml_dtypes

import concourse.bass as bass
import concourse.bacc as bacc
import concourse.mybir as mybir
from concourse.tile import TileContext
from concourse.bass_utils import run_bass_kernel_spmd

P = 4
C = 64
H = W = 256
B = 16
N_CORES = 8
B_LOC = B // N_CORES          # batches per core
WP = W // P                   # patch-cols (64)
F32 = mybir.dt.float32
F16 = mybir.dt.float16
F8 = mybir.dt.float8e4
DR = mybir.MatmulPerfMode.DoubleRow
OSCALE = 1.0 / 32.0
N_WARM = 10

STRIPS = ([(0, 0, 16), (0, 16, 16)]
          + [(0, r, 32) for r in range(32, 256, 32)]
          + [(1, r, 32) for r in range(0, 224, 32)]
          + [(1, 224, 16), (1, 240, 8), (1, 248, 8)])
# column offsets of each strip's xg / x8 block
_goff = [0]
_8off = [0]
for _b, _r0, _rows in STRIPS:
    _hp = _rows // P
    _goff.append(_goff[-1] + 2 * _hp * 192)
    _8off.append(_8off[-1] + 2 * _hp * 64)
TG = _goff[-1]
T8 = _8off[-1]


def _build():
    nc = bacc.Bacc()
    xg = nc.declare_dram_parameter("xg", [128, TG], F16, isOutput=False)
    x8 = nc.declare_dram_parameter("x8", [128, T8], F8, isOutput=False)
    w16 = nc.declare_dram_parameter("w16", [128, 6144], F16, isOutput=False)
    w8 = nc.declare_dram_parameter("w8", [128, 2048], F8, isOutput=False)
    y = nc.declare_dram_parameter("y", [B_LOC, C, H, W], F16, isOutput=True)

    with TileContext(nc) as tc:
        with (
            tc.tile_pool(name="wpool", bufs=1) as wpool,
            tc.tile_pool(name="wupool", bufs=1) as wupool,
            tc.tile_pool(name="gpool", bufs=8) as gpool,
            tc.tile_pool(name="g8pool", bufs=8) as g8pool,
            tc.tile_pool(name="spool", bufs=8) as spool,
            tc.tile_pool(name="psum", bufs=8, space="PSUM") as ppool,
        ):
            # Weights m-major: w16 column j = (m*6 + a*3 + pwi)*128 + (php,c')
            # so each output group's 6 fp16 chunks are contiguous.
            wt = wpool.tile([128, 6144], F16, tag="w")
            w8t = wpool.tile([128, 2048], F8, tag="w8")
            nc.scalar.dma_start(out=w8t[:, 0:1024], in_=w8[:, 0:1024])
            for j in range(12):
                eng = nc.sync if j % 2 == 0 else nc.scalar
                eng.dma_start(out=wt[:, j * 512:(j + 1) * 512],
                              in_=w16[:, j * 512:(j + 1) * 512])
                if j == 3:
                    nc.scalar.dma_start(out=w8t[:, 1024:2048],
                                        in_=w8[:, 1024:2048])
            w8v = w8t[:].rearrange("p (m a f) -> p m a f", m=8, a=2)

            # Warmup: dummy matmuls on a zeroed tile keep the PE busy while
            # the first weights/x land, so HAM un-throttles to 2.4 GHz before
            # real work starts.  The scratch PSUM tile is never read.
            wu = wupool.tile([128, 512], F16, tag="wu")
            nc.vector.memset(wu[:], 0.0)
            wps = ppool.tile([128, 512], F32, tag="ps", name="warm_ps")
            for k in range(N_WARM):
                nc.tensor.matmul(wps[:], lhsT=wu[:, :128], rhs=wu[:],
                                 start=(k == 0), stop=(k == N_WARM - 1))

            for si, (b, r0, rows) in enumerate(STRIPS):
                hp_s = rows // P
                n_l = hp_s * WP
                gt = gpool.tile([128, 2 * hp_s * 192], F16, tag="xg")
                nc.gpsimd.dma_start(out=gt[:],
                                    in_=xg[:, _goff[si]:_goff[si + 1]])
                g8 = g8pool.tile([128, 2 * n_l], F8, tag="x8")
                nc.gpsimd.dma_start(out=g8[:],
                                    in_=x8[:, _8off[si]:_8off[si + 1]])
                g8r = g8[:].rearrange("p (a n) -> p a n", a=2)
                xr = [[gt[:, (a * 3 + pwi) * n_l:(a * 3 + pwi + 1) * n_l]
                       for pwi in range(3)] for a in range(2)]

                dsty4 = y[b, :, r0:r0 + rows, :].rearrange(
                    "c (hp ph) w -> ph c hp w", ph=P)
                for b2 in range(2):
                    st = spool.tile([128, hp_s * 256], F16, tag="st")
                    st_r = st[:].rearrange("p (hp wp pw) -> pw p (hp wp)",
                                           wp=WP, pw=P)
                    for pwp in range(P):
                        m_idx = b2 * P + pwp
                        ps = ppool.tile([128, n_l], F32)
                        # DoubleRow mid-group: both group boundaries stay
                        # fp16<->fp16 (cheap), and start/stop stay on fp16
                        # matmuls (start=True on a DoubleRow measurably
                        # degrades accuracy on hardware)
                        step = 0
                        for a in range(2):
                            for pwi in range(3):
                                f0 = (m_idx * 6 + a * 3 + pwi) * 128
                                nc.tensor.matmul(
                                    ps[:],
                                    lhsT=wt[:, f0:f0 + 128],
                                    rhs=xr[a][pwi],
                                    start=(step == 0),
                                    stop=(step == 5),
                                )
                                step += 1
                            if a == 0:
                                nc.tensor.matmul(
                                    ps[:],
                                    lhsT=w8v[:, m_idx],
                                    rhs=g8r,
                                    start=False,
                                    stop=False,
                                    perf_mode=DR,
                                )
                        if pwp % 2 == 0:
                            nc.scalar.mul(out=st_r[pwp], in_=ps[:], mul=OSCALE)
                        else:
                            nc.vector.tensor_scalar_mul(out=st_r[pwp],
                                                        in0=ps[:],
                                                        scalar1=OSCALE)
                    # output DMAs alternate between the two HWDGE rings
                    for php_off in range(2):
                        srcs = st[php_off * 64:(php_off + 1) * 64, :].rearrange(
                            "p (hp w) -> p hp w", w=256)
                        eng = nc.sync if b2 == 0 else nc.scalar
                        eng.dma_start(out=dsty4[2 * b2 + php_off], in_=srcs)
    nc.compile()
    return nc


def _pack_w(W_mat):
    # All weights packed at 32x so the e4m3 copy sits in its normal range;
    # the PSUM->SBUF copies divide by 32 (exact).
    # lhsT partitions p = ph_off*64 + c over the d-chunk
    # d = c*16 + (2a+ph_off)*4 + pw; e = c'*16 + (2*b2+php_off)*4 + pwp.
    W32 = np.asarray(W_mat, dtype=np.float32) * np.float32(32.0)
    Wr = W32.reshape(64, 2, 2, 4, 64, 2, 2, 4)
    # axes in: (c', b2, php_off, pwp, c, a, ph_off, pw)
    Wp = Wr.transpose(6, 4, 1, 3, 5, 7, 2, 0)
    # -> (ph_off, c, b2, pwp, a, pw, php_off, c')   [m-major columns]
    w16 = np.ascontiguousarray(
        Wp[:, :, :, :, :, 1:4].reshape(128, 6144).astype(np.float16))
    w8 = np.ascontiguousarray(
        Wp[:, :, :, :, :, 0].reshape(128, 2048).astype(ml_dtypes.float8_e4m3fn))
    return w16, w8


def _pack_x(xc):
    # xc: [B_LOC, C, H, W] f32 for one core -> (xg [128,TG] f16, x8 [128,T8] f8)
    x16 = xc.astype(np.float16)
    xg = np.empty((128, TG), dtype=np.float16)
    x8 = np.empty((128, T8), dtype=ml_dtypes.float8_e4m3fn)
    for si, (b, r0, rows) in enumerate(STRIPS):
        hp_s = rows // P
        blk = x16[b, :, r0:r0 + rows, :].reshape(C, hp_s, 2, 2, WP, P)
        # axes: c, hp, a, ph_off, wp, pw -> partitions (ph_off, c)
        t = blk.transpose(2, 3, 0, 5, 1, 4)   # a, pho, c, pw, hp, wp
        g = t[:, :, :, 1:4]                   # a, pho, c, pw', hp, wp
        xg[:, _goff[si]:_goff[si + 1]] = (
            g.transpose(1, 2, 0, 3, 4, 5).reshape(128, -1))
        g8 = t[:, :, :, 0]                    # a, pho, c, hp, wp
        x8[:, _8off[si]:_8off[si + 1]] = (
            g8.transpose(1, 2, 0, 3, 4).reshape(128, -1)
            .astype(ml_dtypes.float8_e4m3fn))
    return xg, x8


_nc_cache = None


def _get_nc():
    global _nc_cache
    if _nc_cache is None:
        _nc_cache = _build()
    return _nc_cache


def _run(x, W_mat, trace=False, **kwargs):
    x = np.ascontiguousarray(np.asarray(x, dtype=np.float32))
    w16, w8 = _pack_w(W_mat)
    nc = _get_nc()
    in_maps = []
    for i in range(N_CORES):
        xg, x8 = _pack_x(x[i * B_LOC:(i + 1) * B_LOC])
        in_maps.append({"xg": xg, "x8": x8, "w16": w16, "w8": w8})
    res = run_bass_kernel_spmd(nc, in_maps, list(range(N_CORES)), trace=trace,
                               **kwargs)
    y = np.concatenate([np.asarray(res.results[i]["y"]).astype(np.float32)
                        for i in range(N_CORES)], axis=0)
    return y, res


def kernel(**inputs):
    y, _ = _run(inputs["x"], inputs["W_mat"])
    return y


# revision 17
# speedup vs baseline: 1.0817x; 1.0292x over previous
"""Patch-orthogonal-mix (unfold -> [L,D]@[D,D]^T -> fold) on 8 Trainium2 NeuronCores.

Strategy: pure data parallel over batch (2 images per core), weights replicated.
Per core, each image is processed in horizontal strips (16 pixel rows for the
first two / last few, 32 for the rest; small edge strips shorten pipeline fill
and drain).

The unfold is done ON THE HOST during input packing: x is cast f32->f16 (the
same RNE rounding the previous in-DMA cast applied) and laid out per-strip in
exactly the SBUF tile format the matmuls consume:
  * xg  [128, .] f16 : partitions p = ph_off*64 + c, free dim (a, pw-1, hp, wp)
        for the six pw!=0 K-chunks (a = row-pair, pw = in-patch column),
  * x8  [128, .] e4m3: the two pw==0 chunks, the DoubleRow moving operand.
This more than halves input HBM traffic (14.7MB vs 33.5MB f32 per core) and
removes the on-device DVE gather stage entirely.  The output is written f16
(16.8MB vs 33.5MB) and upcast to f32 on the host -- the f16 rounding adds
~2.9e-4 relative error in quadrature, invisible next to the 1.70e-2 fp8 term.
With both changes the kernel runs far below the ~358 GB/s HBM-per-core limit
(measured pegged at ~350 with f32 I/O, which stalled the PE mid-run) and is
purely PE-bound.

Mixed-precision contraction: of the 8 K-chunks, the two pw==0 chunks are
computed in fp8-e4m3 through a single DoubleRow matmul (2x MAC rate), the
remaining 6 in fp16: measured ~1526ns per 7-matmul output group vs 1728ns for
8 fp16 passes at a 1.70e-2 relative error.  All weights are host-packed at
32x scale so the e4m3 copy of W stays in its normal range; the PSUM->SBUF
copies then apply the exact 1/32 scale for free.

Schedule: weights are packed m-major (output-tile-major) and sliced into
512-col DMAs (1KB descriptors, same as the input stream's) alternating
between the sync and scalar HWDGE rings -- equal descriptor sizes keep the
SDMA packet round-robin fair so the early weight burst cannot starve the
input stream.  ~10 warmup matmuls on a zeroed tile run while the first data
loads so the PE's HAM clock-gate reaches K=8/8 (2.4 GHz) before real work.
Output DMAs alternate rings; fp32 PSUM accumulation; the fold is realized by
stride-4 interleaving scaled PSUM->SBUF copies (alternating scalar/vector
engines) plus the output DMA pattern.
"""
import numpy as np
import ml_dtypes

import concourse.bass as bass
import concourse.bacc as bacc
import concourse.mybir as mybir
from concourse.tile import TileContext
from concourse.bass_utils import run_bass_kernel_spmd

P = 4
C = 64
H = W = 256
B = 16
N_CORES = 8
B_LOC = B // N_CORES          # batches per core
WP = W // P                   # patch-cols (64)
F32 = mybir.dt.float32
F16 = mybir.dt.float16
F8 = mybir.dt.float8e4
DR = mybir.MatmulPerfMode.DoubleRow
OSCALE = 1.0 / 32.0
N_WARM = 7

STRIPS = ([(0, 0, 16), (0, 16, 16)]
          + [(0, r, 32) for r in range(32, 256, 32)]
          + [(1, r, 32) for r in range(0, 224, 32)]
          + [(1, 224, 16), (1, 240, 8), (1, 248, 8)])
# column offsets of each strip's xg / x8 block
_goff = [0]
_8off = [0]
for _b, _r0, _rows in STRIPS:
    _hp = _rows // P
    _goff.append(_goff[-1] + 2 * _hp * 192)
    _8off.append(_8off[-1] + 2 * _hp * 64)
TG = _goff[-1]
T8 = _8off[-1]


def _build():
    nc = bacc.Bacc()
    xg = nc.declare_dram_parameter("xg", [128, TG], F16, isOutput=False)
    x8 = nc.declare_dram_parameter("x8", [128, T8], F8, isOutput=False)
    w16 = nc.declare_dram_parameter("w16", [128, 6144], F16, isOutput=False)
    w8 = nc.declare_dram_parameter("w8", [128, 2048], F8, isOutput=False)
    y = nc.declare_dram_parameter("y", [B_LOC, C, H, W], F16, isOutput=True)

    with TileContext(nc) as tc:
        with (
            tc.tile_pool(name="wpool", bufs=1) as wpool,
            tc.tile_pool(name="wupool", bufs=1) as wupool,
            tc.tile_pool(name="gpool", bufs=8) as gpool,
            tc.tile_pool(name="g8pool", bufs=8) as g8pool,
            tc.tile_pool(name="spool", bufs=8) as spool,
            tc.tile_pool(name="psum", bufs=8, space="PSUM") as ppool,
        ):
            # Weights m-major: w16 column j = (m*6 + a*3 + pwi)*128 + (php,c')
            # so each output group's 6 fp16 chunks are contiguous.
            wt = wpool.tile([128, 6144], F16, tag="w")
            w8t = wpool.tile([128, 2048], F8, tag="w8")
            # With the slim f16/f8 input stream, HBM has ample slack: load
            # weights as three big DMAs split across the two HWDGE rings.
            nc.scalar.dma_start(out=w8t[:], in_=w8[:])
            nc.sync.dma_start(out=wt[:, 0:3072], in_=w16[:, 0:3072])
            nc.sync.dma_start(out=wt[:, 3072:6144], in_=w16[:, 3072:6144])
            w8v = w8t[:].rearrange("p (m a f) -> p m a f", m=8, a=2)

            # Warmup: dummy matmuls on a zeroed tile keep the PE busy while
            # the first weights/x land, so HAM un-throttles to 2.4 GHz before
            # real work starts.  The scratch PSUM tile is never read.
            wu = wupool.tile([128, 512], F16, tag="wu")
            nc.vector.memset(wu[:], 0.0)
            wps = ppool.tile([128, 512], F32, tag="ps", name="warm_ps")
            for k in range(N_WARM):
                nc.tensor.matmul(wps[:], lhsT=wu[:, :128], rhs=wu[:],
                                 start=(k == 0), stop=(k == N_WARM - 1))

            for si, (b, r0, rows) in enumerate(STRIPS):
                hp_s = rows // P
                n_l = hp_s * WP
                gt = gpool.tile([128, 2 * hp_s * 192], F16, tag="xg")
                nc.gpsimd.dma_start(out=gt[:],
                                    in_=xg[:, _goff[si]:_goff[si + 1]])
                g8 = g8pool.tile([128, 2 * n_l], F8, tag="x8")
                nc.gpsimd.dma_start(out=g8[:],
                                    in_=x8[:, _8off[si]:_8off[si + 1]])
                g8r = g8[:].rearrange("p (a n) -> p a n", a=2)
                xr = [[gt[:, (a * 3 + pwi) * n_l:(a * 3 + pwi + 1) * n_l]
                       for pwi in range(3)] for a in range(2)]

                dsty4 = y[b, :, r0:r0 + rows, :].rearrange(
                    "c (hp ph) w -> ph c hp w", ph=P)
                for b2 in range(2):
                    st = spool.tile([128, hp_s * 256], F16, tag="st")
                    st_r = st[:].rearrange("p (hp wp pw) -> pw p (hp wp)",
                                           wp=WP, pw=P)
                    for pwp in range(P):
                        m_idx = b2 * P + pwp
                        ps = ppool.tile([128, n_l], F32)
                        # DoubleRow mid-group: both group boundaries stay
                        # fp16<->fp16 (cheap), and start/stop stay on fp16
                        # matmuls (start=True on a DoubleRow measurably
                        # degrades accuracy on hardware)
                        step = 0
                        for a in range(2):
                            for pwi in range(3):
                                f0 = (m_idx * 6 + a * 3 + pwi) * 128
                                nc.tensor.matmul(
                                    ps[:],
                                    lhsT=wt[:, f0:f0 + 128],
                                    rhs=xr[a][pwi],
                                    start=(step == 0),
                                    stop=(step == 5),
                                )
                                step += 1
                            if a == 0:
                                nc.tensor.matmul(
                                    ps[:],
                                    lhsT=w8v[:, m_idx],
                                    rhs=g8r,
                                    start=False,
                                    stop=False,
                                    perf_mode=DR,
                                )
                        if pwp % 2 == 0:
                            nc.scalar.mul(out=st_r[pwp], in_=ps[:], mul=OSCALE)
                        else:
                            nc.vector.tensor_scalar_mul(out=st_r[pwp],
                                                        in0=ps[:],
                                                        scalar1=OSCALE)
                    # output DMAs alternate between the two HWDGE rings
                    for php_off in range(2):
                        srcs = st[php_off * 64:(php_off + 1) * 64, :].rearrange(
                            "p (hp w) -> p hp w", w=256)
                        eng = nc.sync if b2 == 0 else nc.scalar
                        eng.dma_start(out=dsty4[2 * b2 + php_off], in_=srcs)
    nc.compile()
    return nc


def _pack_w(W_mat):
    # All weights packed at 32x so the e4m3 copy sits in its normal range;
    # the PSUM->SBUF copies divide by 32 (exact).
    # lhsT partitions p = ph_off*64 + c over the d-chunk
    # d = c*16 + (2a+ph_off)*4 + pw; e = c'*16 + (2*b2+php_off)*4 + pwp.
    W32 = np.asarray(W_mat, dtype=np.float32) * np.float32(32.0)
    Wr = W32.reshape(64, 2, 2, 4, 64, 2, 2, 4)
    # axes in: (c', b2, php_off, pwp, c, a, ph_off, pw)
    Wp = Wr.transpose(6, 4, 1, 3, 5, 7, 2, 0)
    # -> (ph_off, c, b2, pwp, a, pw, php_off, c')   [m-major columns]
    w16 = np.ascontiguousarray(
        Wp[:, :, :, :, :, 1:4].reshape(128, 6144).astype(np.float16))
    w8 = np.ascontiguousarray(
        Wp[:, :, :, :, :, 0].reshape(128, 2048).astype(ml_dtypes.float8_e4m3fn))
    return w16, w8


def _pack_x(xc):
    # xc: [B_LOC, C, H, W] f32 for one core -> (xg [128,TG] f16, x8 [128,T8] f8)
    x16 = xc.astype(np.float16)
    xg = np.empty((128, TG), dtype=np.float16)
    x8 = np.empty((128, T8), dtype=ml_dtypes.float8_e4m3fn)
    for si, (b, r0, rows) in enumerate(STRIPS):
        hp_s = rows // P
        blk = x16[b, :, r0:r0 + rows, :].reshape(C, hp_s, 2, 2, WP, P)
        # axes: c, hp, a, ph_off, wp, pw -> partitions (ph_off, c)
        t = blk.transpose(2, 3, 0, 5, 1, 4)   # a, pho, c, pw, hp, wp
        g = t[:, :, :, 1:4]                   # a, pho, c, pw', hp, wp
        xg[:, _goff[si]:_goff[si + 1]] = (
            g.transpose(1, 2, 0, 3, 4, 5).reshape(128, -1))
        g8 = t[:, :, :, 0]                    # a, pho, c, hp, wp
        x8[:, _8off[si]:_8off[si + 1]] = (
            g8.transpose(1, 2, 0, 3, 4).reshape(128, -1)
            .astype(ml_dtypes.float8_e4m3fn))
    return xg, x8


_nc_cache = None


def _get_nc():
    global _nc_cache
    if _nc_cache is None:
        _nc_cache = _build()
    return _nc_cache


def _run(x, W_mat, trace=False, **kwargs):
    x = np.ascontiguousarray(np.asarray(x, dtype=np.float32))
    w16, w8 = _pack_w(W_mat)
    nc = _get_nc()
    in_maps = []
    for i in range(N_CORES):
        xg, x8 = _pack_x(x[i * B_LOC:(i + 1) * B_LOC])
        in_maps.append({"xg": xg, "x8": x8, "w16": w16, "w8": w8})
    res = run_bass_kernel_spmd(nc, in_maps, list(range(N_CORES)), trace=trace,
                               **kwargs)
    y = np.concatenate([np.asarray(res.results[i]["y"]).astype(np.float32)
                        for i in range(N_CORES)], axis=0)
    return y, res


def kernel(**inputs):
    y, _ = _run(inputs["x"], inputs["W_mat"])
    return y
